# revision 1
# baseline (speedup 1.0000x reference)
"""Multi-head attention forward (B=2, T=2048, C=1024, H=16) on 8 trn2 cores.

Sharding: 2-way data parallel over batch x 4-way tensor parallel over heads
(Megatron-style). Core r handles batch r//4 and heads 4*(r%4)..4*(r%4)+3.
Each core computes Q/K/V projections for its heads, causal flash-style
attention in a transposed (S^T) layout, and its partial c_proj contribution
y_part^T = Wc[:, my_cols] @ o_part^T; partials are reduced on the host.

Device-side layout notes:
- Everything consumed by the PE is float32r (fp22 mantissa truncation,
  full-rate matmul for moving free-dim >= 256).
- x, W are fed pre-transposed and partition-packed by the host so every
  DMA is one big contiguous transfer.
- Softmax is computed without max subtraction (scores are O(12), safe in
  f32) and the denominator comes from an appended ones column in the PV
  stationary operand (V_aug [128, 65]).
"""
import sys

sys.path.insert(0, "/opt/trn_rl_repo")
sys.path.insert(0, "/root/.axon_site")

import numpy as np
import concourse.bacc as bacc
import concourse.mybir as mybir
from concourse import tile
from concourse.bass_utils import run_bass_kernel_spmd

_dt = mybir.dt
F32 = _dt.float32
F32R = _dt.float32r
AF = mybir.ActivationFunctionType
ALU = mybir.AluOpType

B, T, C = 2, 2048, 1024
H, DH = 16, 64
N_CORES = 8
TP = 4              # tensor-parallel width (heads)
HPC = H // TP       # 4 heads per core
CPC = HPC * DH      # 256 channel dims per core
NCH = C // 128      # 8 contraction chunks of 128
QH = T // 2         # 1024-wide q halves
VSTRIDE = (T // 128) * (DH + 1)   # 16 chunks * 65 cols per head in vaug


def _chunks(q0, q1):
    """Split [q0, q1) at 512-aligned boundaries (PSUM-bank safe)."""
    out = []
    c = q0
    while c < q1:
        ce = min(q1, (c // 512 + 1) * 512)
        out.append((c, ce))
        c = ce
    return out


def _build():
    nc = bacc.Bacc("TRN2", target_bir_lowering=False, debug=False,
                   num_devices=N_CORES)

    xt = nc.dram_tensor("xt", [128, NCH * T], F32R, kind="ExternalInput")
    wq = nc.dram_tensor("wq", [128, NCH * CPC], F32R, kind="ExternalInput")
    wk = nc.dram_tensor("wk", [128, NCH * CPC], F32R, kind="ExternalInput")
    wv = nc.dram_tensor("wv", [128, NCH * CPC], F32R, kind="ExternalInput")
    wc = nc.dram_tensor("wc", [128, 2 * C], F32R, kind="ExternalInput")
    msk = nc.dram_tensor("msk", [128, 128], F32, kind="ExternalInput")
    onesd = nc.dram_tensor("onesd", [128, 64], F32R, kind="ExternalInput")
    yt = nc.dram_tensor("yt", [C, T], F32, kind="ExternalOutput")

    with tile.TileContext(nc) as tc:
        with (
            tc.tile_pool(name="sb", bufs=1) as sb,
            tc.tile_pool(name="pt", bufs=3) as ptp,
            tc.tile_pool(name="bcp", bufs=2) as bcp,
            tc.tile_pool(name="yts", bufs=2) as ysb,
            tc.tile_pool(name="mm", bufs=2, space="PSUM") as psA,
            tc.tile_pool(name="ot", bufs=2, space="PSUM") as psO,
        ):
            # ---- loads -------------------------------------------------
            xt_t = sb.tile([128, NCH * T], F32R, tag="xt", name="xt_t")
            nc.sync.dma_start(xt_t[:], xt[:])
            wq_t = sb.tile([128, NCH * CPC], F32R, tag="wq", name="wq_t")
            nc.sync.dma_start(wq_t[:], wq[:])
            wk_t = sb.tile([128, NCH * CPC], F32R, tag="wk", name="wk_t")
            nc.sync.dma_start(wk_t[:], wk[:])
            wv_t = sb.tile([128, NCH * CPC], F32R, tag="wv", name="wv_t")
            nc.sync.dma_start(wv_t[:], wv[:])
            wc_t = sb.tile([128, 2 * C], F32R, tag="wc", name="wc_t")
            nc.sync.dma_start(wc_t[:], wc[:])
            msk_t = sb.tile([128, 128], F32, tag="msk", name="msk_t")
            nc.sync.dma_start(msk_t[:], msk[:])
            ones1 = sb.tile([1, 64], F32R, tag="ones1", name="ones1")
            nc.sync.dma_start(ones1[:], onesd[0:1, :])

            vaug = sb.tile([128, HPC * VSTRIDE], F32R, tag="vaug", name="vaug")
            nc.sync.dma_start(vaug[:, 64::65], onesd[:])

            # ---- Q^T / K^T projections (per head pair) -----------------
            # QT[p] rows: head 2p at partitions 0-63, head 2p+1 at 64-127.
            QT = [sb.tile([128, T], F32R, tag=f"qt{p}", name=f"QT{p}") for p in range(2)]
            KT = [sb.tile([128, T], F32R, tag=f"kt{p}", name=f"KT{p}") for p in range(2)]
            for w_t, dst in ((wq_t, QT), (wk_t, KT)):
                for p in range(2):
                    for ts in range(4):
                        ps = psA.tile([128, 512], F32, tag="mm", name="psmm")
                        for cc in range(NCH):
                            nc.tensor.matmul(
                                ps[:],
                                w_t[:, CPC * cc + 128 * p: CPC * cc + 128 * p + 128],
                                xt_t[:, T * cc + 512 * ts: T * cc + 512 * ts + 512],
                                start=(cc == 0), stop=(cc == NCH - 1),
                            )
                        nc.vector.tensor_copy(dst[p][:, 512 * ts: 512 * ts + 512], ps[:])

            # ---- V (natural [t, d]) into gapped V_aug ------------------
            vaug_h = vaug.rearrange("p (h x) -> p h x", h=HPC)
            for ki in range(T // 128):
                ps = psA.tile([128, CPC], F32, tag="mm", name="psv")
                for cc in range(NCH):
                    nc.tensor.matmul(
                        ps[:],
                        xt_t[:, T * cc + 128 * ki: T * cc + 128 * ki + 128],
                        wv_t[:, CPC * cc: CPC * cc + CPC],
                        start=(cc == 0), stop=(cc == NCH - 1),
                    )
                nc.vector.tensor_copy(
                    vaug_h[:, :, 65 * ki: 65 * ki + 64],
                    ps.rearrange("p (h j) -> p h j", h=HPC),
                )

            # ---- attention (S^T layout, causal, unsafe softmax) --------
            # oTs[p]: normalized o^T for heads 2p (rows 0-63), 2p+1 (64-127)
            oTs = [sb.tile([128, T], F32R, tag=f"ots{p}", name=f"oTs{p}") for p in range(2)]
            for hp in range(2):
                for qh in range(2):
                    kmax = 8 * qh + 8
                    oT = [psO.tile([65, QH], F32, tag="ot", name="oT") for _ in range(2)]
                    for ki in range(kmax):
                        q0 = max(QH * qh, 128 * ki)
                        q1 = QH * (qh + 1)
                        off = q0 - QH * qh
                        for hh in range(2):
                            base = 64 * hh
                            head = 2 * hp + hh
                            st = psA.tile([128, QH], F32, tag="mm", name="st")
                            for (c0, c1) in _chunks(q0, q1):
                                nc.tensor.matmul(
                                    st[:, c0 - QH * qh: c1 - QH * qh],
                                    KT[hp][base:base + 64, 128 * ki:128 * ki + 128],
                                    QT[hp][base:base + 64, c0:c1],
                                    start=True, stop=True,
                                )
                            if 128 * ki >= QH * qh:
                                nc.vector.tensor_add(
                                    st[:, off:off + 128], st[:, off:off + 128], msk_t[:])
                            pt = ptp.tile([128, QH], F32R, tag="pt", name="pt")
                            nc.scalar.activation(
                                pt[:, off:], st[:, off:], AF.Exp, scale=0.125)
                            for (c0, c1) in _chunks(q0, q1):
                                lc0, lc1 = c0 - QH * qh, c1 - QH * qh
                                nc.tensor.matmul(
                                    oT[hh][:, lc0:lc1],
                                    vaug[:, VSTRIDE * head + 65 * ki:
                                         VSTRIDE * head + 65 * ki + 65],
                                    pt[:, lc0:lc1],
                                    start=(ki == 0), stop=(ki == kmax - 1),
                                    skip_group_check=True,
                                )
                    # normalize: o^T[d, q] / denom[q]
                    for hh in range(2):
                        rc = bcp.tile([1, QH], F32R, tag="rc", name="rc")
                        with nc.allow_low_precision(reason="f32r softmax denom"):
                            nc.vector.reciprocal(rc[:], oT[hh][64:65, :])
                        pbc = psA.tile([64, QH], F32, tag="mm", name="pbc")
                        for s0 in range(0, QH, 512):
                            nc.tensor.matmul(pbc[:, s0:s0 + 512], ones1[:],
                                             rc[:, s0:s0 + 512], start=True, stop=True)
                        bcs = bcp.tile([64, QH], F32, tag="bcs", name="bcs")
                        nc.vector.tensor_copy(bcs[:], pbc[:])
                        nc.vector.tensor_tensor(
                            oTs[hp][64 * hh:64 * hh + 64, QH * qh:QH * qh + QH],
                            oT[hh][0:64, :], bcs[:], ALU.mult)

            # ---- partial c_proj: y_part^T = Wc[:, mine].T-chunks @ o^T --
            for dc in range(NCH):
                yt_s = ysb.tile([128, T], F32, tag="yt", name="yt_s")
                for ts in range(4):
                    ps = psA.tile([128, 512], F32, tag="mm", name="psmm")
                    for cc in range(2):
                        nc.tensor.matmul(
                            ps[:],
                            wc_t[:, C * cc + 128 * dc: C * cc + 128 * dc + 128],
                            oTs[cc][:, 512 * ts: 512 * ts + 512],
                            start=(cc == 0), stop=(cc == 1),
                        )
                    nc.vector.tensor_copy(yt_s[:, 512 * ts: 512 * ts + 512], ps[:])
                nc.sync.dma_start(yt[128 * dc: 128 * dc + 128, :], yt_s[:])

    nc.compile()
    return nc


_NC = None


def _get_nc():
    global _NC
    if _NC is None:
        _NC = _build()
    return _NC


def _pack(a):
    """[K*128, n] -> [128, K*n] with row-chunk i at cols [n*i, n*(i+1))."""
    k = a.shape[0] // 128
    return np.ascontiguousarray(
        a.reshape(k, 128, a.shape[1]).transpose(1, 0, 2).reshape(128, -1))


def make_in_maps(x, Wq, Wk, Wv, Wc):
    x = np.asarray(x, np.float32)
    Wq, Wk, Wv, Wc = (np.asarray(w, np.float32) for w in (Wq, Wk, Wv, Wc))
    a = np.arange(128)
    msk = np.where(a[:, None] > a[None, :], np.float32(-1e9), np.float32(0.0))
    onesd = np.ones((128, 64), np.float32)
    xt_b = [_pack(np.ascontiguousarray(x[b].T)) for b in range(B)]
    maps = []
    for r in range(N_CORES):
        b, rho = r // TP, r % TP
        hs = CPC * rho
        maps.append({
            "xt": xt_b[b],
            "wq": _pack(np.ascontiguousarray(Wq[hs:hs + CPC, :].T)),
            "wk": _pack(np.ascontiguousarray(Wk[hs:hs + CPC, :].T)),
            "wv": _pack(np.ascontiguousarray(Wv[hs:hs + CPC, :].T)),
            "wc": _pack(np.ascontiguousarray(Wc[:, hs:hs + CPC].T)),
            "msk": msk,
            "onesd": onesd,
        })
    return maps


def assemble(results, bc):
    bc = np.asarray(bc, np.float32)
    outs = []
    for b in range(B):
        ysum = results[TP * b]["yt"].copy()
        for rho in range(1, TP):
            ysum += results[TP * b + rho]["yt"]
        outs.append(ysum.T + bc[None, :])
    return np.stack(outs).astype(np.float32)


def kernel(x, Wq, Wk, Wv, Wc, bc, _run_kwargs=None):
    nc = _get_nc()
    in_maps = make_in_maps(x, Wq, Wk, Wv, Wc)
    res = run_bass_kernel_spmd(nc, in_maps, core_ids=list(range(N_CORES)),
                               **(_run_kwargs or {}))
    out = assemble(res.results, bc)
    kernel.last_results = res
    return out



# revision 9
# speedup vs baseline: 1.9236x; 1.9236x over previous
"""Multi-head attention forward (B=2, T=2048, C=1024, H=16) on 8 trn2 cores.

Sharding: 2-way data parallel over batch x 4-way tensor parallel over heads
(Megatron-style). Core r handles batch r//4 and heads 4*(r%4)..4*(r%4)+3.
Each core computes Q/K/V projections for its heads, causal attention in a
transposed (S^T) layout, and its partial c_proj contribution
y_part^T = Wc[:, my_cols] @ o_part^T; partials are reduced on the host.

v2 layout/schedule notes:
- All matmul operands are bf16 (1 cycle/row at any moving size, FWL weight
  loads, half DMA traffic); PSUM accumulation stays f32.
- Input DMA is chunked per contraction block so projections start while x
  streams in; Q runs under the x DMA with 8 concurrent PSUM groups.
- Attention is processed in (head-pair, 512-col q-block) phases. Within a
  phase the ki loop is software-pipelined: PV for block ki-1 issues after
  the scores matmul for block ki, so the PE never sits behind the ACT exp.
- Softmax is unnormalized-exp; the denominator is the ones-column of the
  V_aug stationary operand. 1/denom comes from reciprocal_approx_fast and
  is broadcast into the unused partitions 64..127 of the same oT PSUM tile
  via a tile-positioned 1-row f32r matmul (no extra PSUM bank, no serial
  6.5us DVE reciprocal).
- Causal mask = multiplicative binary bf16 mask on the exp output in SBUF.
- PSUM->SBUF eviction copies alternate between DVE and ACT.
"""
import sys

sys.path.insert(0, "/opt/trn_rl_repo")
sys.path.insert(0, "/root/.axon_site")

import ml_dtypes
import numpy as np
import concourse.bacc as bacc
import concourse.mybir as mybir
from concourse import tile
from concourse.bass_utils import run_bass_kernel_spmd

_dt = mybir.dt
F32 = _dt.float32
F32R = _dt.float32r
BF16 = _dt.bfloat16
AF = mybir.ActivationFunctionType
ALU = mybir.AluOpType
BF = ml_dtypes.bfloat16

B, T, C = 2, 2048, 1024
H, DH = 16, 64
N_CORES = 8
TP = 4              # tensor-parallel width (heads)
HPC = H // TP       # 4 heads per core
CPC = HPC * DH      # 256 channel dims per core
NCH = C // 128      # 8 contraction chunks of 128
NKB = T // 128      # 16 key blocks
QB = 512            # q block width in attention
NQB = T // QB       # 4
VSTRIDE = NKB * (DH + 1)   # 16 chunks * 65 cols per head in vaug


def _build():
    nc = bacc.Bacc("TRN2", target_bir_lowering=False, debug=False,
                   num_devices=N_CORES)

    xt = nc.dram_tensor("xt", [128, NCH * T], BF16, kind="ExternalInput")
    wq = nc.dram_tensor("wq", [128, NCH * CPC], BF16, kind="ExternalInput")
    wk = nc.dram_tensor("wk", [128, NCH * CPC], BF16, kind="ExternalInput")
    wv = nc.dram_tensor("wv", [128, NCH * CPC], BF16, kind="ExternalInput")
    wc = nc.dram_tensor("wc", [128, 2 * C], BF16, kind="ExternalInput")
    mskb = nc.dram_tensor("mskb", [128, 128], BF16, kind="ExternalInput")
    onesb = nc.dram_tensor("onesb", [128, 64], BF16, kind="ExternalInput")
    yt = nc.dram_tensor("yt", [C, T], BF16, kind="ExternalOutput")

    with tile.TileContext(nc) as tc:
        with (
            tc.tile_pool(name="sb", bufs=1) as sb,
            tc.tile_pool(name="pt", bufs=3) as ptp,
            tc.tile_pool(name="rcp", bufs=2) as rcp,
            tc.tile_pool(name="yts", bufs=2) as ysb,
        ):
            # ---- loads (chunked so compute can chase the DMA) ----------
            wq_t = sb.tile([128, NCH * CPC], BF16, tag="wq", name="wq_t")
            nc.sync.dma_start(wq_t[:], wq[:])
            xt_t = sb.tile([128, NCH * T], BF16, tag="xt", name="xt_t")
            for cc in range(NCH):
                nc.sync.dma_start(xt_t[:, T * cc: T * cc + T],
                                  xt[:, T * cc: T * cc + T])
            wk_t = sb.tile([128, NCH * CPC], BF16, tag="wk", name="wk_t")
            nc.sync.dma_start(wk_t[:], wk[:])
            wv_t = sb.tile([128, NCH * CPC], BF16, tag="wv", name="wv_t")
            nc.sync.dma_start(wv_t[:], wv[:])
            wc_t = sb.tile([128, 2 * C], BF16, tag="wc", name="wc_t")
            nc.sync.dma_start(wc_t[:], wc[:])
            mskb_t = sb.tile([128, 128], BF16, tag="mskb", name="mskb_t")
            nc.sync.dma_start(mskb_t[:], mskb[:])
            ones1_t = sb.tile([1, 64], BF16, tag="ones1", name="ones1_t")
            nc.sync.dma_start(ones1_t[:], onesb[0:1, :])
            vaug = sb.tile([128, HPC * VSTRIDE], BF16, tag="vaug", name="vaug")
            nc.sync.dma_start(vaug[:, 64::65], onesb[:])

            QT = [sb.tile([128, T], BF16, tag=f"qt{p}", name=f"QT{p}") for p in range(2)]
            KT = [sb.tile([128, T], BF16, tag=f"kt{p}", name=f"KT{p}") for p in range(2)]
            oTs = [sb.tile([128, T], BF16, tag=f"ots{p}", name=f"oTs{p}") for p in range(2)]
            vaug_h = vaug.rearrange("p (h x) -> p h x", h=HPC)

            cp_i = [0]

            def evict(dst_ap, src_ap):
                """PSUM -> SBUF copy, alternating DVE / ACT."""
                cp_i[0] += 1
                if cp_i[0] % 2:
                    nc.vector.tensor_copy(dst_ap, src_ap)
                else:
                    nc.scalar.copy(dst_ap, src_ap)

            # ---- projections -------------------------------------------
            with tc.tile_pool(name="ps8", bufs=1, space="PSUM") as ps8:
                # Q: 8 concurrent groups, contraction chunk outer so the
                # matmuls chase the x DMA chunk by chunk.
                for w_t, dst in ((wq_t, QT), (wk_t, KT)):
                    psG = [ps8.tile([128, 512], F32, tag=f"g{i}", name=f"ps{i}")
                           for i in range(8)]
                    for cc in range(NCH):
                        for i in range(8):
                            p, ts = i // 4, i % 4
                            nc.tensor.matmul(
                                psG[i][:],
                                w_t[:, CPC * cc + 128 * p: CPC * cc + 128 * p + 128],
                                xt_t[:, T * cc + 512 * ts: T * cc + 512 * ts + 512],
                                start=(cc == 0), stop=(cc == NCH - 1),
                            )
                    for i in range(8):
                        p, ts = i // 4, i % 4
                        evict(dst[p][:, 512 * ts: 512 * ts + 512], psG[i][:])

                # V (natural [t, d]) into gapped V_aug
                for rnd in range(2):
                    psV = [ps8.tile([128, 512], F32, tag=f"g{i}", name=f"psv{i}")
                           for i in range(8)]
                    for cc in range(NCH):
                        for i in range(8):
                            ki = 8 * rnd + i
                            nc.tensor.matmul(
                                psV[i][:, 0:CPC],
                                xt_t[:, T * cc + 128 * ki: T * cc + 128 * ki + 128],
                                wv_t[:, CPC * cc: CPC * cc + CPC],
                                start=(cc == 0), stop=(cc == NCH - 1),
                            )
                    for i in range(8):
                        ki = 8 * rnd + i
                        evict(
                            vaug_h[:, :, 65 * ki: 65 * ki + 64],
                            psV[i][:, 0:CPC].rearrange("p (h j) -> p h j", h=HPC),
                        )

            # ---- attention (S^T layout, causal, unsafe softmax) --------
            with (
                tc.tile_pool(name="st", bufs=2, space="PSUM") as stp,
                tc.tile_pool(name="ot", bufs=2, space="PSUM") as otp,
            ):
                pending_norm = []

                def flush_norm():
                    for (hp_, qb_, hh_, oT_) in pending_norm:
                        dns = rcp.tile([1, QB], F32, tag="dns", name="dns")
                        nc.vector.tensor_copy(dns[:], oT_[64:65, :])
                        rc = rcp.tile([1, QB], F32, tag="rc", name="rc")
                        rcb = rcp.tile([1, QB], BF16, tag="rcb", name="rcb")
                        nc.vector.reciprocal_approx_fast(rc[:], dns[:])
                        nc.vector.tensor_copy(rcb[:], rc[:])
                        # broadcast 1/denom into partitions 64..127 of oT
                        nc.tensor.matmul(
                            oT_[64:128, :], ones1_t[:], rcb[:],
                            start=True, stop=True, skip_group_check=True,
                        )
                        bcs = rcp.tile([64, QB], BF16, tag="bcs", name="bcs")
                        nc.vector.tensor_copy(bcs[:], oT_[64:128, :])
                        nc.vector.tensor_tensor(
                            oTs[hp_][64 * hh_: 64 * hh_ + 64,
                                     QB * qb_: QB * qb_ + QB],
                            oT_[0:64, :], bcs[:], ALU.mult)
                    pending_norm.clear()

                for hp in range(2):
                    for qb in range(NQB):
                        q0b = QB * qb
                        kmax = 4 * qb + 4
                        oT = [otp.tile([128, QB], F32, tag=f"o{hh}", name=f"oT{hh}")
                              for hh in range(2)]
                        prev = None
                        for ki in range(kmax):
                            off = max(0, 128 * ki - q0b)
                            st = stp.tile([128, 2 * QB], F32, tag="st", name="st")
                            for hh in range(2):
                                base = 64 * hh
                                nc.tensor.matmul(
                                    st[:, QB * hh + off: QB * (hh + 1)],
                                    KT[hp][base:base + 64, 128 * ki:128 * ki + 128],
                                    QT[hp][base:base + 64, q0b + off: q0b + QB],
                                    start=True, stop=True,
                                )
                            pt = ptp.tile([128, 2 * QB], BF16, tag="pt", name="pt")
                            stv = st.rearrange("p (h q) -> p h q", h=2)[:, :, off:]
                            ptv = pt.rearrange("p (h q) -> p h q", h=2)[:, :, off:]
                            nc.scalar.activation(ptv, stv, AF.Exp, scale=0.125)
                            if 128 * ki >= q0b:  # diagonal block: apply mask
                                for hh in range(2):
                                    d0 = QB * hh + off
                                    nc.vector.tensor_tensor(
                                        pt[:, d0:d0 + 128], pt[:, d0:d0 + 128],
                                        mskb_t[:], ALU.mult)
                            if ki == 1:
                                flush_norm()
                            if prev is not None:
                                pki, ppt, poff = prev
                                for hh in range(2):
                                    head = 2 * hp + hh
                                    nc.tensor.matmul(
                                        oT[hh][0:65, poff:QB],
                                        vaug[:, VSTRIDE * head + 65 * pki:
                                             VSTRIDE * head + 65 * pki + 65],
                                        ppt[:, QB * hh + poff: QB * (hh + 1)],
                                        start=(pki == 0), stop=(pki == kmax - 1),
                                        skip_group_check=True,
                                    )
                            prev = (ki, pt, off)
                        pki, ppt, poff = prev
                        for hh in range(2):
                            head = 2 * hp + hh
                            nc.tensor.matmul(
                                oT[hh][0:65, poff:QB],
                                vaug[:, VSTRIDE * head + 65 * pki:
                                     VSTRIDE * head + 65 * pki + 65],
                                ppt[:, QB * hh + poff: QB * (hh + 1)],
                                start=(pki == 0), stop=(pki == kmax - 1),
                                skip_group_check=True,
                            )
                        for hh in range(2):
                            pending_norm.append((hp, qb, hh, oT[hh]))
                flush_norm()

            # ---- partial c_proj: y_part^T = Wc[:, mine].T-chunks @ o^T --
            with tc.tile_pool(name="psy", bufs=4, space="PSUM") as psY:
                for dc in range(NCH):
                    yt_s = ysb.tile([128, T], BF16, tag="yt", name="yt_s")
                    for ts in range(4):
                        ps = psY.tile([128, 512], F32, tag="y", name="psy")
                        for cc in range(2):
                            nc.tensor.matmul(
                                ps[:],
                                wc_t[:, C * cc + 128 * dc: C * cc + 128 * dc + 128],
                                oTs[cc][:, 512 * ts: 512 * ts + 512],
                                start=(cc == 0), stop=(cc == 1),
                            )
                        evict(yt_s[:, 512 * ts: 512 * ts + 512], ps[:])
                    nc.sync.dma_start(yt[128 * dc: 128 * dc + 128, :], yt_s[:])

    nc.compile()
    return nc


_NC = None


def _get_nc():
    global _NC
    if _NC is None:
        _NC = _build()
    return _NC


def _pack(a):
    """[K*128, n] -> [128, K*n] with row-chunk i at cols [n*i, n*(i+1))."""
    k = a.shape[0] // 128
    return np.ascontiguousarray(
        a.reshape(k, 128, a.shape[1]).transpose(1, 0, 2).reshape(128, -1))


def make_in_maps(x, Wq, Wk, Wv, Wc):
    x = np.asarray(x, np.float32).astype(BF)
    Wq, Wk, Wv, Wc = (np.asarray(w, np.float32).astype(BF)
                      for w in (Wq, Wk, Wv, Wc))
    a = np.arange(128)
    mskb = (a[:, None] <= a[None, :]).astype(BF)
    onesb = np.ones((128, 64), BF)
    xt_b = [_pack(np.ascontiguousarray(x[b].T)) for b in range(B)]
    maps = []
    for r in range(N_CORES):
        b, rho = r // TP, r % TP
        hs = CPC * rho
        maps.append({
            "xt": xt_b[b],
            "wq": _pack(np.ascontiguousarray(Wq[hs:hs + CPC, :].T)),
            "wk": _pack(np.ascontiguousarray(Wk[hs:hs + CPC, :].T)),
            "wv": _pack(np.ascontiguousarray(Wv[hs:hs + CPC, :].T)),
            "wc": _pack(np.ascontiguousarray(Wc[:, hs:hs + CPC].T)),
            "mskb": mskb,
            "onesb": onesb,
        })
    return maps


def assemble(results, bc):
    bc = np.asarray(bc, np.float32)
    outs = []
    for b in range(B):
        ysum = results[TP * b]["yt"].astype(np.float32)
        for rho in range(1, TP):
            ysum += results[TP * b + rho]["yt"].astype(np.float32)
        outs.append(ysum.T + bc[None, :])
    return np.stack(outs).astype(np.float32)


def kernel(x, Wq, Wk, Wv, Wc, bc, _run_kwargs=None):
    nc = _get_nc()
    in_maps = make_in_maps(x, Wq, Wk, Wv, Wc)
    res = run_bass_kernel_spmd(nc, in_maps, core_ids=list(range(N_CORES)),
                               **(_run_kwargs or {}))
    out = assemble(res.results, bc)
    kernel.last_results = res
    return out


# revision 14
# speedup vs baseline: 2.0138x; 1.0469x over previous
"""Multi-head attention forward (B=2, T=2048, C=1024, H=16) on 8 trn2 cores.

Sharding: 2-way data parallel over batch x 4-way tensor parallel over heads
(Megatron-style). Core r handles batch r//4 and heads 4*(r%4)..4*(r%4)+3.
Each core computes Q/K/V projections for its heads, causal attention in a
transposed (S^T) layout, and its partial c_proj contribution
y_part^T = Wc[:, my_cols] @ o_part^T; partials are reduced on the host.

v2 layout/schedule notes:
- All matmul operands are bf16 (1 cycle/row at any moving size, FWL weight
  loads, half DMA traffic); PSUM accumulation stays f32.
- Input DMA is chunked per contraction block so projections start while x
  streams in; Q runs under the x DMA with 8 concurrent PSUM groups.
- Attention is processed in (head-pair, 512-col q-block) phases. Within a
  phase the ki loop is software-pipelined: PV for block ki-1 issues after
  the scores matmul for block ki, so the PE never sits behind the ACT exp.
- Softmax is unnormalized-exp; the denominator is the ones-column of the
  V_aug stationary operand. 1/denom comes from reciprocal_approx_fast and
  is broadcast into the unused partitions 64..127 of the same oT PSUM tile
  via a tile-positioned 1-row f32r matmul (no extra PSUM bank, no serial
  6.5us DVE reciprocal).
- Causal mask = multiplicative binary bf16 mask on the exp output in SBUF.
- PSUM->SBUF eviction copies alternate between DVE and ACT.
"""
import sys

sys.path.insert(0, "/opt/trn_rl_repo")
sys.path.insert(0, "/root/.axon_site")

import ml_dtypes
import numpy as np
import concourse.bacc as bacc
import concourse.mybir as mybir
from concourse import tile
from concourse.bass_utils import run_bass_kernel_spmd

_dt = mybir.dt
F32 = _dt.float32
F32R = _dt.float32r
BF16 = _dt.bfloat16
AF = mybir.ActivationFunctionType
ALU = mybir.AluOpType
BF = ml_dtypes.bfloat16

B, T, C = 2, 2048, 1024
H, DH = 16, 64
N_CORES = 8
TP = 4              # tensor-parallel width (heads)
HPC = H // TP       # 4 heads per core
CPC = HPC * DH      # 256 channel dims per core
NCH = C // 128      # 8 contraction chunks of 128
NKB = T // 128      # 16 key blocks
QB = 512            # q block width in attention
NQB = T // QB       # 4
VSTRIDE = NKB * (DH + 1)   # 16 chunks * 65 cols per head in vaug


def _build():
    nc = bacc.Bacc("TRN2", target_bir_lowering=False, debug=False,
                   num_devices=N_CORES)

    xt = nc.dram_tensor("xt", [128, NCH * T], BF16, kind="ExternalInput")
    wq = nc.dram_tensor("wq", [128, NCH * CPC], BF16, kind="ExternalInput")
    wk = nc.dram_tensor("wk", [128, NCH * CPC], BF16, kind="ExternalInput")
    wv = nc.dram_tensor("wv", [128, NCH * CPC], BF16, kind="ExternalInput")
    wc = nc.dram_tensor("wc", [128, 2 * C], BF16, kind="ExternalInput")
    mskb = nc.dram_tensor("mskb", [128, 128], BF16, kind="ExternalInput")
    onesb = nc.dram_tensor("onesb", [128, 64], BF16, kind="ExternalInput")
    yt = nc.dram_tensor("yt", [C, T], BF16, kind="ExternalOutput")

    with tile.TileContext(nc) as tc:
        with (
            tc.tile_pool(name="sb", bufs=1) as sb,
            tc.tile_pool(name="pt", bufs=4) as ptp,
            tc.tile_pool(name="rcp", bufs=2) as rcp,
            tc.tile_pool(name="yts", bufs=2) as ysb,
        ):
            # ---- loads (chunked so compute can chase the DMA) ----------
            wq_t = sb.tile([128, NCH * CPC], BF16, tag="wq", name="wq_t")
            nc.sync.dma_start(wq_t[:], wq[:])
            # one tile per contraction chunk so deps are per-chunk
            xt_c = []
            for cc in range(NCH):
                xc = sb.tile([128, T], BF16, tag=f"xt{cc}", name=f"xt{cc}")
                nc.sync.dma_start(xc[:], xt[:, T * cc: T * cc + T])
                xt_c.append(xc)
            wk_t = sb.tile([128, NCH * CPC], BF16, tag="wk", name="wk_t")
            nc.sync.dma_start(wk_t[:], wk[:])
            wv_t = sb.tile([128, NCH * CPC], BF16, tag="wv", name="wv_t")
            nc.sync.dma_start(wv_t[:], wv[:])
            wc_t = sb.tile([128, 2 * C], BF16, tag="wc", name="wc_t")
            nc.sync.dma_start(wc_t[:], wc[:])
            mskb_t = sb.tile([128, 128], BF16, tag="mskb", name="mskb_t")
            nc.sync.dma_start(mskb_t[:], mskb[:])
            ones1_t = sb.tile([1, 64], BF16, tag="ones1", name="ones1_t")
            nc.sync.dma_start(ones1_t[:], onesb[0:1, :])
            vaug = sb.tile([128, HPC * VSTRIDE], BF16, tag="vaug", name="vaug")
            nc.sync.dma_start(vaug[:, 64::65], onesb[:])

            QT = [sb.tile([128, T], BF16, tag=f"qt{p}", name=f"QT{p}") for p in range(2)]
            KT = [sb.tile([128, T], BF16, tag=f"kt{p}", name=f"KT{p}") for p in range(2)]
            oTs = [sb.tile([128, T], BF16, tag=f"ots{p}", name=f"oTs{p}") for p in range(2)]
            vaug_h = vaug.rearrange("p (h x) -> p h x", h=HPC)

            cp_i = [0]

            def evict(dst_ap, src_ap):
                """PSUM -> SBUF copy, alternating DVE / ACT."""
                cp_i[0] += 1
                if cp_i[0] % 2:
                    nc.vector.tensor_copy(dst_ap, src_ap)
                else:
                    nc.scalar.copy(dst_ap, src_ap)

            # ---- projections -------------------------------------------
            with tc.tile_pool(name="ps8", bufs=1, space="PSUM") as ps8:
                # Q: 8 concurrent groups, contraction chunk outer so the
                # matmuls chase the x DMA chunk by chunk.
                for w_t, dst in ((wq_t, QT), (wk_t, KT)):
                    psG = [ps8.tile([128, 512], F32, tag=f"g{i}", name=f"ps{i}")
                           for i in range(8)]
                    for cc in range(NCH):
                        for i in range(8):
                            p, ts = i // 4, i % 4
                            nc.tensor.matmul(
                                psG[i][:],
                                w_t[:, CPC * cc + 128 * p: CPC * cc + 128 * p + 128],
                                xt_c[cc][:, 512 * ts: 512 * ts + 512],
                                start=(cc == 0), stop=(cc == NCH - 1),
                            )
                    for i in range(8):
                        p, ts = i // 4, i % 4
                        evict(dst[p][:, 512 * ts: 512 * ts + 512], psG[i][:])

                # V (natural [t, d]) into gapped V_aug
                for rnd in range(2):
                    psV = [ps8.tile([128, 512], F32, tag=f"g{i}", name=f"psv{i}")
                           for i in range(8)]
                    for cc in range(NCH):
                        for i in range(8):
                            ki = 8 * rnd + i
                            nc.tensor.matmul(
                                psV[i][:, 0:CPC],
                                xt_c[cc][:, 128 * ki: 128 * ki + 128],
                                wv_t[:, CPC * cc: CPC * cc + CPC],
                                start=(cc == 0), stop=(cc == NCH - 1),
                            )
                    for i in range(8):
                        ki = 8 * rnd + i
                        evict(
                            vaug_h[:, :, 65 * ki: 65 * ki + 64],
                            psV[i][:, 0:CPC].rearrange("p (h j) -> p h j", h=HPC),
                        )

            # ---- attention (S^T layout, causal, unsafe softmax) --------
            with (
                tc.tile_pool(name="st", bufs=2, space="PSUM") as stp,
                tc.tile_pool(name="ot", bufs=2, space="PSUM") as otp,
            ):
                pending_norm = []

                def flush_norm():
                    for (hp_, qb_, hh_, oT_) in pending_norm:
                        dns = rcp.tile([1, QB], F32, tag="dns", name="dns")
                        nc.vector.tensor_copy(dns[:], oT_[64:65, :])
                        rc = rcp.tile([1, QB], F32, tag="rc", name="rc")
                        rcb = rcp.tile([1, QB], BF16, tag="rcb", name="rcb")
                        nc.vector.reciprocal_approx_fast(rc[:], dns[:])
                        nc.vector.tensor_copy(rcb[:], rc[:])
                        # broadcast 1/denom into partitions 64..127 of oT
                        nc.tensor.matmul(
                            oT_[64:128, :], ones1_t[:], rcb[:],
                            start=True, stop=True, skip_group_check=True,
                        )
                        bcs = rcp.tile([64, QB], BF16, tag="bcs", name="bcs")
                        nc.vector.tensor_copy(bcs[:], oT_[64:128, :])
                        nc.vector.tensor_tensor(
                            oTs[hp_][64 * hh_: 64 * hh_ + 64,
                                     QB * qb_: QB * qb_ + QB],
                            oT_[0:64, :], bcs[:], ALU.mult)
                    pending_norm.clear()

                for hp in range(2):
                    for qb in range(NQB):
                        q0b = QB * qb
                        kmax = 4 * qb + 4
                        oT = [otp.tile([128, QB], F32, tag=f"o{hh}", name=f"oT{hh}")
                              for hh in range(2)]

                        def issue_pv(entry, oT=oT, hp=hp, kmax=kmax):
                            pki, ppt, poff = entry
                            for hh in range(2):
                                head = 2 * hp + hh
                                nc.tensor.matmul(
                                    oT[hh][0:65, poff:QB],
                                    vaug[:, VSTRIDE * head + 65 * pki:
                                         VSTRIDE * head + 65 * pki + 65],
                                    ppt[:, QB * hh + poff: QB * (hh + 1)],
                                    start=(pki == 0), stop=(pki == kmax - 1),
                                    skip_group_check=True,
                                )

                        pend = []
                        for ki in range(kmax):
                            off = max(0, 128 * ki - q0b)
                            st = stp.tile([128, 2 * QB], F32, tag="st", name="st")
                            for hh in range(2):
                                base = 64 * hh
                                nc.tensor.matmul(
                                    st[:, QB * hh + off: QB * (hh + 1)],
                                    KT[hp][base:base + 64, 128 * ki:128 * ki + 128],
                                    QT[hp][base:base + 64, q0b + off: q0b + QB],
                                    start=True, stop=True,
                                )
                            pt = ptp.tile([128, 2 * QB], BF16, tag="pt", name="pt")
                            stv = st.rearrange("p (h q) -> p h q", h=2)[:, :, off:]
                            ptv = pt.rearrange("p (h q) -> p h q", h=2)[:, :, off:]
                            nc.scalar.activation(ptv, stv, AF.Exp, scale=0.125)
                            if 128 * ki >= q0b:  # diagonal block: apply mask
                                for hh in range(2):
                                    d0 = QB * hh + off
                                    nc.vector.tensor_tensor(
                                        pt[:, d0:d0 + 128], pt[:, d0:d0 + 128],
                                        mskb_t[:], ALU.mult)
                            if ki == 1:
                                flush_norm()
                            if len(pend) >= 2:
                                issue_pv(pend.pop(0))
                            pend.append((ki, pt, off))
                        for entry in pend:
                            issue_pv(entry)
                        for hh in range(2):
                            pending_norm.append((hp, qb, hh, oT[hh]))
                flush_norm()

            # ---- partial c_proj: y_part^T = Wc[:, mine].T-chunks @ o^T --
            with tc.tile_pool(name="psy", bufs=4, space="PSUM") as psY:
                for dc in range(NCH):
                    yt_s = ysb.tile([128, T], BF16, tag="yt", name="yt_s")
                    for ts in range(4):
                        ps = psY.tile([128, 512], F32, tag="y", name="psy")
                        for cc in range(2):
                            nc.tensor.matmul(
                                ps[:],
                                wc_t[:, C * cc + 128 * dc: C * cc + 128 * dc + 128],
                                oTs[cc][:, 512 * ts: 512 * ts + 512],
                                start=(cc == 0), stop=(cc == 1),
                            )
                        evict(yt_s[:, 512 * ts: 512 * ts + 512], ps[:])
                    nc.sync.dma_start(yt[128 * dc: 128 * dc + 128, :], yt_s[:])

    nc.compile()
    return nc


_NC = None


def _get_nc():
    global _NC
    if _NC is None:
        _NC = _build()
    return _NC


def _pack(a):
    """[K*128, n] -> [128, K*n] with row-chunk i at cols [n*i, n*(i+1))."""
    k = a.shape[0] // 128
    return np.ascontiguousarray(
        a.reshape(k, 128, a.shape[1]).transpose(1, 0, 2).reshape(128, -1))


def make_in_maps(x, Wq, Wk, Wv, Wc):
    x = np.asarray(x, np.float32).astype(BF)
    Wq, Wk, Wv, Wc = (np.asarray(w, np.float32).astype(BF)
                      for w in (Wq, Wk, Wv, Wc))
    a = np.arange(128)
    mskb = (a[:, None] <= a[None, :]).astype(BF)
    onesb = np.ones((128, 64), BF)
    xt_b = [_pack(np.ascontiguousarray(x[b].T)) for b in range(B)]
    maps = []
    for r in range(N_CORES):
        b, rho = r // TP, r % TP
        hs = CPC * rho
        maps.append({
            "xt": xt_b[b],
            "wq": _pack(np.ascontiguousarray(Wq[hs:hs + CPC, :].T)),
            "wk": _pack(np.ascontiguousarray(Wk[hs:hs + CPC, :].T)),
            "wv": _pack(np.ascontiguousarray(Wv[hs:hs + CPC, :].T)),
            "wc": _pack(np.ascontiguousarray(Wc[:, hs:hs + CPC].T)),
            "mskb": mskb,
            "onesb": onesb,
        })
    return maps


def assemble(results, bc):
    bc = np.asarray(bc, np.float32)
    outs = []
    for b in range(B):
        ysum = results[TP * b]["yt"].astype(np.float32)
        for rho in range(1, TP):
            ysum += results[TP * b + rho]["yt"].astype(np.float32)
        outs.append(ysum.T + bc[None, :])
    return np.stack(outs).astype(np.float32)


def kernel(x, Wq, Wk, Wv, Wc, bc, _run_kwargs=None):
    nc = _get_nc()
    in_maps = make_in_maps(x, Wq, Wk, Wv, Wc)
    res = run_bass_kernel_spmd(nc, in_maps, core_ids=list(range(N_CORES)),
                               **(_run_kwargs or {}))
    out = assemble(res.results, bc)
    kernel.last_results = res
    return out
